# revision 9
# baseline (speedup 1.0000x reference)
"""Trainium2 Bass kernel for nn_Event_Critic_Net (dual-branch GAT critic).

Math: the reference only reads the GAT output at the LAST node of each
graph (graphs are 32 contiguous nodes), so only edges whose dst is a
graph's last node contribute.  For those edges the softmax-weighted
aggregation commutes with the linear projection W:

    out_g = sigmoid( (sum_n alpha[n] * x[n,:]) @ W + bias )
    alpha[n] = cnt[n]*exp(e[n]) / (sum_n cnt[n]*exp(e[n]) + 1e-16)
    e[n] = leaky_relu(x[n]. w_src + x[last(g)]. w_dst),  w_* = W @ att_*

cnt[n] = number of edges (n -> last(g(n))).  Per graph only ~7 distinct
source nodes have cnt>0, so the host COMPACTS each graph to K node slots
(zero-padded); GPT graphs share a 128-partition tile -> T tiles per core
instead of 128.  Graph-structure prep (edge counts, gather, tiling,
transposed copy, weight replication) happens on host; all FLOPs on
device.  Sharding: graphs are data-parallel across the 8 cores.

Device pipeline (phases interleaved across branches to keep PE hot):
  logits : xt-chunk [128,128] stationary (FWL), wv2 [128,2] moving
           -> asps psum [128, 2*NCH] (node-layout, 1 bank)
  a_dst  : xl2 mult+reduce -> transpose -> Qm matmul -> adbc [128,T]
  P-chain: z=asps+adbc, e=leakyrelu(DVE), exp(ACT set0), P=e*cnt
  M-build: one tensor_tensor with to_broadcast: M[p,(t,j)]=P[p,t]*Bm[p,j]
  agg    : per tile t: xg-tile [128,66] stationary, M[:,GPT*t..] moving
           -> ynT psum x2; row 64 = denominator (ones column)
  norm   : ACT-Copy evac, recip(DVE) -> rbc = ones64 (x) recip (matmul)
  proj   : ynrm = y*rbc, Wb [64,128] stationary -> h psum [128,512]
  sigmoid: via exp (set0): eu=exp(-h-b); sg_u*sg_d = 1/((1+eu)(1+ed))
  tail   : q=(1+eu)(1+ed), r=recip(q), mlp matmul -> [1,512]+b -> out
"""

import numpy as np
from contextlib import ExitStack

NC = 8            # cores
N = 131072        # nodes total
G = 4096          # graphs
NPG = 32          # nodes per graph
S = 64            # state size
H = 128           # hidden size
GPC = G // NC     # 512 graphs per core
SA = 66           # xg tile cols: 64 feats | ones | pad

_CACHE = {}


def _layout(K):
    GPT = 128 // K               # graphs per tile
    T = -(-GPC // GPT)           # tiles per core
    NT = T * 128                 # slot-rows per core per branch
    XTC = NT // 2                # xt columns
    NCH = XTC // 128             # logit chunks (NT divisible by 256)
    assert NCH * 128 == XTC
    return GPT, T, NT, XTC, NCH


def _build_module(K):
    import concourse.tile as tile
    from concourse import bacc, mybir
    from concourse.alu_op_type import AluOpType as Alu

    GPT, T, NT, XTC, NCH = _layout(K)
    f32 = mybir.dt.float32
    bf16 = mybir.dt.bfloat16
    Act = mybir.ActivationFunctionType
    AxX = mybir.AxisListType.X

    nc = bacc.Bacc("TRN2", target_bir_lowering=False, debug=False,
                   num_devices=NC)

    dram = {}

    def din(name, shape, dt=f32):
        dram[name] = nc.dram_tensor(name, shape, dt, kind="ExternalInput")

    for p in ("u", "d"):
        din(f"{p}_xg", [128, T * SA], bf16)
        din(f"{p}_xt", [128, XTC], bf16)
        din(f"{p}_cnt", [128, T])
        din(f"{p}_xl2", [128, GPT * S], bf16)
    din("cstf", [128, 205])
    din("cstb", [128, 520], bf16)
    out_dram = nc.dram_tensor("out", [1, GPC], f32, kind="ExternalOutput")

    # chunked loads: xt split at 128-col multiples, xg at SA-col multiples
    def split5(n):
        a = max(1, n // 12)
        b = (n - a) // 4
        return [0, a, a + b, a + 2 * b, a + 3 * b, n]
    XT_SPLIT = split5(NCH)
    XG_SPLIT = split5(T)

    with tile.TileContext(nc) as tc, ExitStack() as ctx:
        const = ctx.enter_context(tc.tile_pool(name="const", bufs=1))
        xp = ctx.enter_context(tc.tile_pool(name="xp", bufs=2))
        wk = ctx.enter_context(tc.tile_pool(name="wk", bufs=2))
        ps1 = ctx.enter_context(tc.tile_pool(name="ps1", bufs=1, space="PSUM"))
        psA = ctx.enter_context(tc.tile_pool(name="psA", bufs=2, space="PSUM"))
        psY = ctx.enter_context(tc.tile_pool(name="psY", bufs=2, space="PSUM"))

        cstf = const.tile([128, 205], f32, tag="cstf")
        nc.gpsimd.dma_start(cstf[:], dram["cstf"].ap())
        cstb = const.tile([128, 520], bf16, tag="cstb")
        nc.gpsimd.dma_start(cstb[:], dram["cstb"].ap())
        ident = cstf[:, 0:128]
        eps = cstf[0:1, 128:129]
        mlpb = cstf[0:1, 129:130]
        nbias = {"u": cstf[:, 130:131], "d": cstf[:, 131:132]}  # negated
        Bmf = cstf[:, 132:132 + GPT]
        ones64 = cstf[0:1, 140:204]
        wv2s = {"u": cstb[:, 0:2], "d": cstb[:, 2:4]}
        wdsts = {"u": cstb[:, 4:68], "d": cstb[:, 68:132]}
        Qm = cstb[0:GPT, 132:260]
        Ws = {"u": cstb[0:S, 260:388], "d": cstb[0:S, 388:516]}
        mlpW = cstb[:, 516:517]
        mlpWf = cstf[:, 204:205]

        st = {"u": {}, "d": {}}
        # ---- big loads: ALL on the Sync HWDGE ring, in consumption
        # order (xt-u, xt-d, xg-u, xg-d) so arrivals chase the PE ----
        for p in ("u", "d"):
            s = st[p]
            s["xt"] = []
            for i in range(5):
                w = (XT_SPLIT[i + 1] - XT_SPLIT[i]) * 128
                t_ = xp.tile([128, w], bf16, tag=f"xt{i}", name=f"xt{i}_{p}")
                nc.sync.dma_start(t_[:], dram[f"{p}_xt"].ap()[
                    :, XT_SPLIT[i] * 128:XT_SPLIT[i + 1] * 128])
                s["xt"].append(t_)
        for p in ("u", "d"):
            s = st[p]
            s["xg"] = []
            for i in range(5):
                w = (XG_SPLIT[i + 1] - XG_SPLIT[i]) * SA
                t_ = xp.tile([128, w], bf16, tag=f"xg{i}", name=f"xg{i}_{p}")
                nc.sync.dma_start(t_[:], dram[f"{p}_xg"].ap()[
                    :, XG_SPLIT[i] * SA:XG_SPLIT[i + 1] * SA])
                s["xg"].append(t_)

        for p in ("u", "d"):
            s = st[p]
            cnt = wk.tile([128, T], f32, tag=f"cnt_{p}")
            nc.gpsimd.dma_start(cnt[:], dram[f"{p}_cnt"].ap())
            s["cnt"] = cnt
            xl2 = wk.tile([128, GPT * S], bf16, tag=f"xl2_{p}")
            nc.gpsimd.dma_start(xl2[:], dram[f"{p}_xl2"].ap())
            s["xl2"] = xl2

        # ---- logits (both branches back to back on PE) ----
        for p in ("u", "d"):
            s = st[p]
            asps = psA.tile([128, 2 * NCH], f32, tag="asps", name=f"as_{p}")
            s["asps"] = asps
            for c in range(NCH):
                blk = 0
                while XT_SPLIT[blk + 1] <= c:
                    blk += 1
                cc = c - XT_SPLIT[blk]
                nc.tensor.matmul(
                    asps[:, 2 * c:2 * c + 2],
                    s["xt"][blk][:, 128 * cc:128 * cc + 128],
                    wv2s[p],
                    start=True, stop=True)

        # ---- a_dst chain + P-chain per branch (DVE/ACT overlap PE) ----
        for p in ("u", "d"):
            s = st[p]
            tmp6 = wk.tile([128, GPT * S], f32, tag=f"tmp6_{p}")
            nc.vector.tensor_tensor(
                tmp6[:].rearrange("p (j s) -> p j s", s=S),
                s["xl2"][:].rearrange("p (j s) -> p j s", s=S),
                wdsts[p].unsqueeze(1).to_broadcast((128, GPT, S)),
                op=Alu.mult)
            adst = wk.tile([128, GPT], f32, tag=f"adst_{p}")
            nc.vector.tensor_reduce(
                adst[:], tmp6[:].rearrange("p (j s) -> p j s", s=S),
                axis=AxX, op=Alu.add)
            tp = ps1.tile([GPT, 128], f32, tag="mix", name=f"adT_{p}")
            nc.tensor.transpose(tp[:], adst[:], ident)
            adT = wk.tile([GPT, 128], bf16, tag=f"adTs_{p}")
            nc.vector.tensor_copy(adT[:], tp[:])
            adbc_ps = ps1.tile([128, T], f32, tag="mix", name=f"adbc_{p}")
            nc.tensor.matmul(adbc_ps[:], Qm, adT[:, 0:T],
                             start=True, stop=True)
            adbc = wk.tile([128, T], f32, tag=f"adbcs_{p}")
            nc.vector.tensor_copy(adbc[:], adbc_ps[:])

            z = wk.tile([128, T], f32, tag=f"z_{p}")
            nc.vector.tensor_tensor(
                z[:].rearrange("p (j c) -> p j c", j=2),
                s["asps"][:].rearrange("p (c j) -> p j c", j=2),
                adbc[:].rearrange("p (j c) -> p j c", j=2), op=Alu.add)
            e = wk.tile([128, T], f32, tag=f"e_{p}")
            nc.vector.scalar_tensor_tensor(
                e[:], z[:], 0.2, z[:], op0=Alu.mult, op1=Alu.max)
            ex = wk.tile([128, T], f32, tag=f"ex_{p}")
            nc.scalar.activation(ex[:], e[:], Act.Exp)
            P = wk.tile([128, T], f32, tag=f"P_{p}")
            nc.vector.tensor_tensor(P[:], ex[:], s["cnt"][:], op=Alu.mult)

            M = wk.tile([128, T * GPT], bf16, tag=f"M_{p}")
            nc.vector.tensor_tensor(
                M[:].rearrange("p (t j) -> p t j", j=GPT),
                P[:].unsqueeze(2).to_broadcast((128, T, GPT)),
                Bmf.unsqueeze(1).to_broadcast((128, T, GPT)),
                op=Alu.mult)
            s["M"] = M

        # ---- aggregation (both branches back to back on PE) ----
        for p in ("u", "d"):
            s = st[p]
            yns = wk.tile([SA, T * GPT], f32, tag=f"yns_{p}")
            s["yns"] = yns
            for h in range(2):
                t0, t1 = (0, T // 2) if h == 0 else (T // 2, T)
                ynT = psY.tile([SA, (t1 - t0) * GPT], f32, tag="ynT",
                               name=f"ynT_{p}{h}")
                for t in range(t0, t1):
                    blk = 0
                    while XG_SPLIT[blk + 1] <= t:
                        blk += 1
                    tt = t - XG_SPLIT[blk]
                    nc.tensor.matmul(
                        ynT[:, GPT * (t - t0):GPT * (t - t0 + 1)],
                        s["xg"][blk][:, SA * tt:SA * tt + SA],
                        s["M"][:, GPT * t:GPT * (t + 1)],
                        start=True, stop=True)
                nc.scalar.activation(
                    yns[:, t0 * GPT:t1 * GPT], ynT[:], Act.Copy)

        # ---- normalize + project + exp per branch ----
        for p in ("u", "d"):
            s = st[p]
            yns = s["yns"]
            dn = wk.tile([1, GPC], f32, tag=f"dn_{p}")
            nc.vector.tensor_scalar(
                dn[:], yns[S:S + 1, 0:GPC], eps, None, op0=Alu.add)
            rp = wk.tile([1, GPC], f32, tag=f"rp_{p}")
            nc.vector.reciprocal_approx_fast(rp[:], dn[:])
            rbc = ps1.tile([S, GPC], f32, tag="mix", name=f"rbc_{p}")
            nc.tensor.matmul(rbc[:], ones64, rp[:], start=True, stop=True)
            ynrm = wk.tile([S, GPC], bf16, tag=f"ynrm_{p}")
            nc.vector.tensor_tensor(ynrm[:], yns[0:S, 0:GPC], rbc[:],
                                    op=Alu.mult)
            hT = ps1.tile([H, GPC], f32, tag="hT", name=f"hT_{p}")
            nc.tensor.matmul(hT[:], Ws[p], ynrm[:], start=True, stop=True)
            # eu = exp(-(h + b)) ;  sigmoid(h+b) = 1/(1+eu)
            eu = wk.tile([H, GPC], bf16, tag=f"eu_{p}")
            nc.scalar.activation(eu[:], hT[:], Act.Exp, bias=nbias[p],
                                 scale=-1.0)
            s["eu"] = eu

        # ---- combine: sg_u*sg_d = 1/((1+eu)(1+ed)) ----
        ed1 = wk.tile([H, GPC], bf16, tag="ed1")
        nc.vector.tensor_scalar(
            ed1[:], st["d"]["eu"][:], 1.0, None, op0=Alu.add)
        q = wk.tile([H, GPC], f32, tag="q")
        nc.vector.scalar_tensor_tensor(
            q[:], st["u"]["eu"][:], 1.0, ed1[:], op0=Alu.add, op1=Alu.mult)
        r32 = wk.tile([H, GPC], f32, tag="r32")
        nc.vector.reciprocal_approx_fast(r32[:], q[:])
        o_ps = ps1.tile([1, GPC], f32, tag="mix", name="o_ps")
        nc.tensor.matmul(o_ps[:], mlpWf, r32[:], start=True, stop=True)
        o_sb = wk.tile([1, GPC], f32, tag="o_sb")
        nc.vector.tensor_scalar(
            o_sb[:], o_ps[:], mlpb, None, op0=Alu.add)
        nc.sync.dma_start(out_dram.ap(), o_sb[:])

    nc.compile()
    return nc


def _get_module(K):
    key = ("nc", K)
    if key not in _CACHE:
        _CACHE[key] = _build_module(K)
    return _CACHE[key]


def _branch_meta(ei):
    """nodes/counts/slots for one branch (host, structure only)."""
    src = np.asarray(ei[0]).astype(np.int64)
    dst = np.asarray(ei[1]).astype(np.int64)
    valid = (dst % NPG) == (NPG - 1)
    nodes, counts = np.unique(src[valid], return_counts=True)
    gids = nodes // NPG
    order = np.argsort(gids, kind="stable")
    gs = gids[order]
    first = np.r_[True, gs[1:] != gs[:-1]]
    idx_of_first = np.maximum.accumulate(
        np.where(first, np.arange(len(gs)), 0))
    slot_sorted = np.arange(len(gs)) - idx_of_first
    slot = np.empty(len(nodes), np.int64)
    slot[order] = slot_sorted
    maxd = int(slot.max()) + 1 if slot.size else 0
    return nodes, counts, gids, slot, maxd


def _prep_branch(x, W, att_src, att_dst, meta, K):
    import ml_dtypes
    bf = ml_dtypes.bfloat16
    GPT, T, NT, XTC, NCH = _layout(K)
    x = np.asarray(x, np.float32)
    W = np.asarray(W, np.float32)
    w_src = (W @ np.asarray(att_src, np.float32)).astype(np.float32)
    w_dst = (W @ np.asarray(att_dst, np.float32)).astype(np.float32)
    nodes, counts, gids, slot, _ = meta

    per_core = []
    for c in range(NC):
        g_lo, g_hi = c * GPC, (c + 1) * GPC
        m = (gids >= g_lo) & (gids < g_hi)
        nl, cl, gl, sl = nodes[m], counts[m], gids[m] - g_lo, slot[m]
        t = gl // GPT
        part = (gl % GPT) * K + sl
        xg = np.zeros((128, T, SA), np.float32)
        xg[part, t, :S] = x[nl]
        xg[:, :, S] = 1.0
        xg2 = np.ascontiguousarray(xg.reshape(128, T * SA)).astype(bf)
        cnt_t = np.zeros((128, T), np.float32)
        cnt_t[part, t] = cl.astype(np.float32)
        xflat = np.zeros((NT, S), np.float32)
        xflat[t * 128 + part] = x[nl]
        xtv = xflat.reshape(2, XTC, S).transpose(0, 2, 1)
        xtv = np.ascontiguousarray(xtv.reshape(128, XTC)).astype(bf)
        lg = np.arange(g_lo * NPG + NPG - 1, g_hi * NPG, NPG)
        xl = x[lg].reshape(GPC, S)
        xl2 = np.zeros((128, GPT, S), np.float32)
        gg = np.arange(GPC)
        xl2[gg // GPT, gg % GPT] = xl
        xl2 = np.ascontiguousarray(xl2.reshape(128, GPT * S)).astype(bf)
        per_core.append({"xg": xg2, "xt": xtv, "cnt": cnt_t, "xl2": xl2})

    shared = {"w_src": w_src, "w_dst": w_dst, "W": W}
    return per_core, shared


def _build_in_maps(inputs, metas, K):
    import ml_dtypes
    bf = ml_dtypes.bfloat16
    GPT, T, NT, XTC, NCH = _layout(K)
    pcs = {}
    shareds = {}
    pcs["u"], shareds["u"] = _prep_branch(
        inputs["up_x"], inputs["up_W"],
        inputs["up_att_src"], inputs["up_att_dst"], metas["u"], K)
    pcs["d"], shareds["d"] = _prep_branch(
        inputs["down_x"], inputs["down_W"],
        inputs["down_att_src"], inputs["down_att_dst"], metas["d"], K)

    pp = np.arange(128)
    cstf = np.zeros((128, 205), np.float32)
    cstf[:, 0:128] = np.eye(128, dtype=np.float32)
    cstf[0, 128] = 1e-16
    cstf[0, 129] = float(np.asarray(inputs["mlp_b"]).reshape(-1)[0])
    cstf[:, 130] = -np.asarray(inputs["up_bias"], np.float32)
    cstf[:, 131] = -np.asarray(inputs["down_bias"], np.float32)
    # Bmf [128, GPT]: 1 if p//K == j (p < GPT*K)
    cstf[pp[:GPT * K], 132 + pp[:GPT * K] // K] = 1.0
    cstf[0, 140:204] = 1.0
    cstf[:, 204] = np.asarray(inputs["mlp_W"], np.float32).reshape(H)

    cstb = np.zeros((128, 520), np.float32)
    for i, p in enumerate(("u", "d")):
        ws = shareds[p]["w_src"]
        cstb[0:S, 0 + 2 * i] = ws
        cstb[S:128, 1 + 2 * i] = ws
        cstb[:, 4 + S * i:4 + S * (i + 1)] = np.broadcast_to(
            shareds[p]["w_dst"], (128, S))
    # Qm [GPT, 128]: 1 if m//K == j (m < GPT*K)
    for j in range(GPT):
        cstb[j, 132 + j * K:132 + (j + 1) * K] = 1.0
    cstb[0:S, 260:388] = shareds["u"]["W"]
    cstb[0:S, 388:516] = shareds["d"]["W"]
    cstb[:, 516] = np.asarray(inputs["mlp_W"], np.float32).reshape(H)

    common = {
        "cstf": cstf,
        "cstb": cstb.astype(bf),
    }

    in_maps = []
    for c in range(NC):
        m = dict(common)
        for p in ("u", "d"):
            for k2, v in pcs[p][c].items():
                m[f"{p}_{k2}"] = v
        in_maps.append(m)
    return in_maps


def kernel(**inputs):
    from concourse.bass_utils import run_bass_kernel_spmd

    metas = {"u": _branch_meta(inputs["up_edge_index"]),
             "d": _branch_meta(inputs["down_edge_index"])}
    maxd = max(metas["u"][4], metas["d"][4])
    K = 18 if maxd <= 18 else maxd  # compiled layout adapts to the data
    nc = _get_module(K)
    in_maps = _build_in_maps(inputs, metas, K)
    res = run_bass_kernel_spmd(nc, in_maps, core_ids=list(range(NC)))
    out = np.concatenate(
        [np.asarray(r["out"], np.float32).reshape(GPC) for r in res.results])
    return out.reshape(G, 1)
